# revision 1
# baseline (speedup 1.0000x reference)
"""ColorICP fused single-launch kernel for 8 Trainium2 NeuronCores.

Data-parallel over image rows: each core owns a 120-row band ([120,1280],
partition=row) plus a +-16-row halo window of frame-1 so all bilinear warp
gathers are core-local (measured warp displacement is < 8 rows). The whole
3-iteration Gauss-Newton loop runs in ONE device program: pre-stage packs
vertex1/normal1/x1 into a [152*1280, 9] DRAM record array and the 21 rgb
Gram planes M_t into DRAM; each iteration projects via the current pose,
fetches 4 bilinear taps per pixel with indirect DMA, forms 58 Gram product
planes, reduces, AllReduces the tiny normal equations and solves the 6x6
system (Gauss-Jordan) + SE(3) update replicated on every core.
"""
import numpy as np
import concourse.bass as bass
import concourse.mybir as mybir
import concourse.tile as tile_mod
from concourse.vector_clock import ScopedClock
from concourse.masks import make_identity

f32 = mybir.dt.float32
f16 = mybir.dt.float16
i32 = mybir.dt.int32
OP = mybir.AluOpType
ACT = mybir.ActivationFunctionType

H, W = 960, 1280
NB = 8
BAND = H // NB            # 120
HALO = 16
VW = BAND + 2 * HALO      # 152 window rows
NREC = VW * W
CH = 128                  # chunk columns
NCHUNK = W // CH          # 10
MAX_ITER = 3
DAMPING = 1e-3
HUBER_B = 0.02
LAMBDA_RGB = 1e-6
DIST_THRESH = 0.1

PAIRS = [(a, b) for a in range(6) for b in range(a, 6)]
NENT = 58   # 21 icp JtWJ + 6 icp JtR + 21 rgb JtWJ + 5 P + 5 Q
P_LIST = [("ab", 0, -1.0), ("a2p1", 1, 1.0), ("bpl", 2, -1.0), ("invD", 3, 1.0), ("aID", 5, -1.0)]
Q_LIST = [("b2p1", 0, -1.0), ("ab", 1, 1.0), ("a", 2, 1.0), ("invD", 4, 1.0), ("bID", 5, -1.0)]

CPOSE = 0
CFX, CFY, CCX, CCY = 12, 13, 14, 15
CD1MIN, CD1MAX = 16, 17
CW0, CR0 = 18, 19
CNEG = 20      # 20..25
CEYE9 = 26     # 26..34
CIFX, CIFY = 36, 37
CD0S, CD0O, CD1S, CD1O = 38, 39, 40, 41


def _patched_drain(self, tick_clock, wait_clock):
    probe = self.nc.sync.nop()
    wait_clock.add_sem_waits(probe.ins, ScopedClock({None: tick_clock.global_clock}))
    si = probe.ins.sync_info
    waits = list(si.on_wait)
    si.on_wait.clear()
    id2h = {h.num: h for h in self.sems.allocated().values()}
    for w in waits:
        if w.sync_type == "semaphore" and w.id in id2h:
            self.nc.sync.wait_ge(id2h[w.id], w.wait_value)
        else:
            si.on_wait.append(w)
    self.nc.sync.drain()
    self.nc.all_engine_barrier()
    popped = self.nc._tile_sem_poison_stack.pop()
    assert popped is self._sem_poison
    self.nc.clear_and_free_semaphores(list(self.sems.allocated().values()))
    self.nc.all_engine_barrier()


tile_mod.TileContext._drain_and_barrier = _patched_drain


def _sobel_raw(nc, pr, t0, t1, t2, nr, tagp, name, ctag=""):
    """Unnormalized sobel from 3 partition-aligned row-shifted views of the
    source ([nr, Wl] each: rows r-1, r, r+1). Replicate-padded columns."""
    Wl = t0.shape[1]
    Asm = pr.tile([nr, Wl], f32, tag=tagp + "Asm", name=name + "Asm")
    nc.vector.tensor_tensor(Asm[:], t0, t2, OP.add)
    nc.vector.scalar_tensor_tensor(Asm[:], t1, 2.0, Asm[:], OP.mult, OP.add)
    dx = pr.tile([nr, Wl], f32, tag=tagp + "dx" + ctag, name=name + "dx")
    nc.vector.tensor_tensor(dx[:, 1:Wl - 1], Asm[:, 2:Wl], Asm[:, 0:Wl - 2], OP.subtract)
    nc.vector.tensor_tensor(dx[:, 0:1], Asm[:, 1:2], Asm[:, 0:1], OP.subtract)
    nc.vector.tensor_tensor(dx[:, Wl - 1:Wl], Asm[:, Wl - 1:Wl], Asm[:, Wl - 2:Wl - 1], OP.subtract)
    Bv = pr.tile([nr, Wl], f32, tag=tagp + "Bv", name=name + "Bv")
    nc.vector.tensor_tensor(Bv[:], t2, t0, OP.subtract)
    dy = pr.tile([nr, Wl], f32, tag=tagp + "dy" + ctag, name=name + "dy")
    nc.vector.tensor_tensor(dy[:, 1:Wl - 1], Bv[:, 0:Wl - 2], Bv[:, 2:Wl], OP.add)
    nc.vector.scalar_tensor_tensor(dy[:, 1:Wl - 1], Bv[:, 1:Wl - 1], 2.0, dy[:, 1:Wl - 1], OP.mult, OP.add)
    nc.vector.tensor_tensor(dy[:, 0:1], Bv[:, 0:1], Bv[:, 1:2], OP.add)
    nc.vector.scalar_tensor_tensor(dy[:, 0:1], Bv[:, 0:1], 2.0, dy[:, 0:1], OP.mult, OP.add)
    nc.vector.tensor_tensor(dy[:, Wl - 1:Wl], Bv[:, Wl - 1:Wl], Bv[:, Wl - 2:Wl - 1], OP.add)
    nc.vector.scalar_tensor_tensor(dy[:, Wl - 1:Wl], Bv[:, Wl - 1:Wl], 2.0, dy[:, Wl - 1:Wl], OP.mult, OP.add)
    return dx, dy


def build_program(num_devices=NB):
    nc = bass.Bass(num_devices=num_devices)

    def reg_const(value, dtype=f32):
        t = nc.alloc_sbuf_tensor(f"cst-{dtype.name}-{value}", [128, 1], dtype)
        nc.gpsimd.memset(t.ap(), value)
        nc.const_aps.aps[(dtype, value)] = t.ap()

    reg_const(1e-16)
    reg_const(1e-8)
    nc.all_engine_barrier()

    d_d0 = nc.declare_dram_parameter("d0", [BAND, W], mybir.dt.uint8, isOutput=False)
    d_d1h = nc.declare_dram_parameter("d1h", [VW + 2, W], mybir.dt.uint8, isOutput=False)
    d_x0c = nc.declare_dram_parameter("x0c", [3 * (BAND + 2), W], mybir.dt.uint8, isOutput=False)
    d_x1c = nc.declare_dram_parameter("x1c", [3 * VW, W], mybir.dt.uint8, isOutput=False)
    d_cst = nc.declare_dram_parameter("consts", [128, 64], f32, isOutput=False)
    d_out = nc.declare_dram_parameter("out", [1, 64], f32, isOutput=True)

    d_pix = nc.dram_tensor("pix", [NREC, 9], f32)
    d_rec4 = nc.dram_tensor("rec4", [NREC, 36], f32)
    d_M = nc.dram_tensor("Mpl", [BAND, 21, W], f32)
    d_gram = nc.dram_tensor("gram", [1, 64], f32)
    d_gramS = nc.dram_tensor("gramS", [1, 64], f32, kind="Internal", addr_space="Shared")

    groups = [list(range(num_devices))]

    with tile_mod.TileContext(nc) as tc:
        with tc.tile_pool(name="pers", bufs=1) as pp, \
             tc.tile_pool(name="ps", bufs=1, space="PSUM") as pps:
            # ------------- consts + persistent statics -------------------
            cst = pp.tile([128, 64], f32)
            nc.sync.dma_start(out=cst[:], in_=d_cst.ap())

            def CS(col, n=BAND):
                return cst[0:n, col:col + 1]

            d0 = pp.tile([BAND, W], f32)
            a = pp.tile([128, W], f32)
            bcol = pp.tile([BAND, 1], f32)
            bpl = pp.tile([BAND, W], f32)
            ab = pp.tile([BAND, W], f32)
            invD = pp.tile([BAND, W], f32)
            x0p = [pp.tile([BAND, W], f32, name=f"x0_{c}") for c in range(3)]
            gx = [pp.tile([BAND, W], f32, name=f"gx_{c}") for c in range(3)]
            gy = [pp.tile([BAND, W], f32, name=f"gy_{c}") for c in range(3)]
            ident = pp.tile([BAND, BAND], f32)
            ones120 = pp.tile([1, BAND], f32)
            ones6 = pp.tile([1, 6], f32)
            poseflat = pp.tile([1, 12], f32)
            Rp33 = pp.tile([3, 3], f32)
            pose_bc = pp.tile([BAND, 12], f32)
            acc3 = pp.tile([BAND, NENT * NCHUNK], f32)
            gram_sb = pp.tile([1, 64], f32)

            with tc.tile_pool(name="pre1", bufs=1) as pr:
                d016 = pr.tile([BAND, W], mybir.dt.uint8, tag="ld")
                nc.sync.dma_start(out=d016[:], in_=d_d0.ap())
                nc.vector.tensor_scalar(d0[:], d016[:], CS(CD0S), CS(CD0O), OP.mult, OP.add)
                iL = pr.tile([128, W], i32, tag="ioi")
                nc.gpsimd.iota(iL[:], [[1, W]], base=0, channel_multiplier=0)
                nc.vector.tensor_copy(a[:], iL[:])
                nc.vector.tensor_scalar(a[:], a[:], CS(CCX, 128), CS(CIFX, 128), OP.subtract, OP.mult)
                jL = pr.tile([BAND, 1], i32, tag="ioj")
                nc.gpsimd.iota(jL[:], [[1, 1]], base=0, channel_multiplier=1)
                nc.vector.tensor_copy(bcol[:], jL[:])
                nc.vector.tensor_scalar(bcol[:], bcol[:], CS(CR0), None, OP.add)
                nc.vector.tensor_scalar(bcol[:], bcol[:], CS(CCY), CS(CIFY), OP.subtract, OP.mult)
                onesp = pr.tile([BAND, W], f32, tag="ones")
                nc.vector.memset(onesp[:], 1.0)
                nc.vector.tensor_scalar(bpl[:], onesp[:], bcol[:], None, OP.mult)
                nc.vector.tensor_scalar(ab[:], a[0:BAND, :], bcol[:], None, OP.mult)
                nc.vector.reciprocal(invD[:], d0[:])
                make_identity(nc, ident[:])
                nc.vector.memset(ones120[:], 1.0)
                nc.vector.memset(ones6[:], 1.0)
                nc.vector.tensor_copy(poseflat[:], cst[0:1, 0:12])
                nc.sync.dma_start(out=Rp33[:], in_=poseflat[:, 0:9])
                pose_ps = pps.tile([BAND, 12], f32, tag="poseps")
                nc.tensor.matmul(out=pose_ps[:], lhsT=ones120[:], rhs=poseflat[:])
                nc.vector.tensor_copy(pose_bc[:], pose_ps[:])

                # sobel(x0) normalized, scaled by fx/fy
                for c in range(3):
                    xc16 = pr.tile([BAND + 2, W], mybir.dt.uint8, tag="x0l")
                    nc.sync.dma_start(out=xc16[:], in_=d_x0c.ap()[c * (BAND + 2):(c + 1) * (BAND + 2), :])
                    xc = pr.tile([BAND + 2, W], f32, tag="x0f")
                    nc.vector.tensor_scalar(xc[:], xc16[:], 1.0 / 255.0, None, OP.mult)
                    xs1 = pr.tile([BAND, W], f32, tag="xs1")
                    nc.sync.dma_start(out=xs1[:], in_=xc[1:BAND + 1, :])
                    xs2 = pr.tile([BAND, W], f32, tag="xs2")
                    nc.sync.dma_start(out=xs2[:], in_=xc[2:BAND + 2, :])
                    nc.vector.tensor_copy(x0p[c][:], xs1[:])
                    dx, dy = _sobel_raw(nc, pr, xc[0:BAND, :], xs1[:], xs2[:], BAND, "sb", f"sb{c}")
                    s1 = pr.tile([BAND, W], f32, tag="s1")
                    nc.vector.tensor_tensor(s1[:], dx[:], dx[:], OP.mult)
                    s2 = pr.tile([BAND, W], f32, tag="s2")
                    nc.vector.tensor_tensor(s2[:], dy[:], dy[:], OP.mult)
                    nc.vector.tensor_tensor(s1[:], s1[:], s2[:], OP.add)
                    nc.scalar.activation(s1[:], s1[:], ACT.Sqrt, bias=1e-8)
                    nc.vector.reciprocal(s1[:], s1[:])
                    nc.vector.tensor_tensor(gx[c][:], dx[:], s1[:], OP.mult)
                    nc.vector.tensor_scalar(gx[c][:], gx[c][:], CS(CFX), None, OP.mult)
                    nc.vector.tensor_tensor(gy[c][:], dy[:], s1[:], OP.mult)
                    nc.vector.tensor_scalar(gy[c][:], gy[c][:], CS(CFY), None, OP.mult)

            # ------------- M planes (column halves) ----------------------
            with tc.tile_pool(name="pre2", bufs=1) as pr:
                for hc in range(2):
                    cs, ce = hc * (W // 2), (hc + 1) * (W // 2)
                    Wl = W // 2
                    sgn = {}
                    for nm_, src in [("nab", ab), ("nbpl", bpl), ("ninvD", invD)]:
                        t = pr.tile([BAND, Wl], f32, tag=nm_, name=f"{nm_}_{hc}")
                        nc.vector.tensor_scalar(t[:], src[:, cs:ce], -1.0, None, OP.mult)
                        sgn[nm_] = t
                    a2p1h = pr.tile([BAND, Wl], f32, tag="a2p1h", name=f"a2p1h_{hc}")
                    nc.vector.tensor_tensor(a2p1h[:], a[0:BAND, cs:ce], a[0:BAND, cs:ce], OP.mult)
                    nc.vector.tensor_scalar(a2p1h[:], a2p1h[:], 1.0, None, OP.add)
                    nb2p1h = pr.tile([BAND, Wl], f32, tag="nb2p1h", name=f"nb2p1h_{hc}")
                    nc.vector.tensor_scalar(nb2p1h[:], bpl[:, cs:ce], bcol[:], None, OP.mult)
                    nc.vector.tensor_scalar(nb2p1h[:], nb2p1h[:], 1.0, -1.0, OP.add, OP.mult)
                    naIDh = pr.tile([BAND, Wl], f32, tag="naIDh", name=f"naIDh_{hc}")
                    nc.vector.tensor_tensor(naIDh[:], a[0:BAND, cs:ce], sgn["ninvD"][:], OP.mult)
                    nbIDh = pr.tile([BAND, Wl], f32, tag="nbIDh", name=f"nbIDh_{hc}")
                    nc.vector.tensor_scalar(nbIDh[:], sgn["ninvD"][:], bcol[:], None, OP.mult)
                    jxs = [sgn["nab"], a2p1h, sgn["nbpl"], invD, None, naIDh]
                    jys = [nb2p1h, ab, a, None, invD, nbIDh]

                    def JSL(t, full=False):
                        # full-band planes get sliced; half planes used as-is
                        return t[0:BAND, cs:ce] if t is a else (t[:, cs:ce] if t in (invD, ab) else t[:])

                    T = {}
                    for ci in range(3):
                        for ai in range(6):
                            t = pr.tile([BAND, Wl], f32, tag=f"T{ci}{ai}", name=f"T_{hc}_{ci}_{ai}")
                            if jxs[ai] is None:
                                nc.vector.tensor_tensor(t[:], gy[ci][:, cs:ce], JSL(jys[ai]), OP.mult)
                            elif jys[ai] is None:
                                nc.vector.tensor_tensor(t[:], gx[ci][:, cs:ce], JSL(jxs[ai]), OP.mult)
                            else:
                                nc.vector.tensor_tensor(t[:], gx[ci][:, cs:ce], JSL(jxs[ai]), OP.mult)
                                t2 = pr.tile([BAND, Wl], f32, tag="Tt2")
                                nc.vector.tensor_tensor(t2[:], gy[ci][:, cs:ce], JSL(jys[ai]), OP.mult)
                                nc.vector.tensor_tensor(t[:], t[:], t2[:], OP.add)
                            T[(ci, ai)] = t
                    for ti, (ai, bi) in enumerate(PAIRS):
                        m = pr.tile([BAND, Wl], f32, tag="Mt", bufs=2, name=f"Mt_{hc}_{ti}")
                        nc.vector.tensor_tensor(m[:], T[(0, ai)][:], T[(0, bi)][:], OP.mult)
                        mt = pr.tile([BAND, Wl], f32, tag="Mtt", name=f"Mtt_{hc}_{ti}")
                        for ci in (1, 2):
                            nc.vector.tensor_tensor(mt[:], T[(ci, ai)][:], T[(ci, bi)][:], OP.mult)
                            nc.vector.tensor_tensor(m[:], m[:], mt[:], OP.add)
                        nc.sync.dma_start(out=d_M.ap()[:, ti, cs:ce], in_=m[:])

            # ------------- pix window build ------------------------------
            with tc.tile_pool(name="pre3", bufs=1) as pr:
                for (g0, g1, nr, woff) in [(0, 128, 126, 0), (126, VW + 2, 26, 126)]:
                    gp = g1 - g0
                    d1g16 = pr.tile([gp, W], mybir.dt.uint8, tag="d1l", name=f"d1l_{g0}")
                    nc.sync.dma_start(out=d1g16[:], in_=d_d1h.ap()[g0:g1, :])
                    d1g = pr.tile([gp, W], f32, tag="d1f", name=f"d1f_{g0}")
                    nc.vector.tensor_scalar(d1g[:], d1g16[:], cst[0:gp, CD1S:CD1S + 1], cst[0:gp, CD1O:CD1O + 1], OP.mult, OP.add)
                    jLg = pr.tile([gp, 1], i32, tag="iojg", name=f"iojg_{g0}")
                    nc.gpsimd.iota(jLg[:], [[1, 1]], base=g0 - 1, channel_multiplier=1)
                    byg = pr.tile([gp, 1], f32, tag="byg", name=f"byg_{g0}")
                    nc.vector.tensor_copy(byg[:], jLg[:])
                    nc.vector.tensor_scalar(byg[:], byg[:], cst[0:gp, CW0:CW0 + 1], None, OP.add)
                    nc.vector.tensor_scalar(byg[:], byg[:], cst[0:gp, CCY:CCY + 1], cst[0:gp, CIFY:CIFY + 1], OP.subtract, OP.mult)
                    vx = pr.tile([gp, W], f32, tag="vx", name=f"vx_{g0}")
                    nc.vector.tensor_tensor(vx[:], a[0:gp, :], d1g[:], OP.mult)
                    vy = pr.tile([gp, W], f32, tag="vy", name=f"vy_{g0}")
                    nc.vector.tensor_scalar(vy[:], d1g[:], byg[:], None, OP.mult)
                    vch = [vx, vy, d1g]
                    # partition-aligned row shifts (compute ops must start at
                    # partition 0 -- realize vertical shifts via DMA copies)
                    sh1, sh2 = [], []
                    for c in range(3):
                        s1t = pr.tile([nr, W], f32, tag=f"sh1_{c}", name=f"sh1_{g0}_{c}")
                        nc.sync.dma_start(out=s1t[:], in_=vch[c][1:nr + 1, :])
                        s2t = pr.tile([nr, W], f32, tag=f"sh2_{c}", name=f"sh2_{g0}_{c}")
                        nc.sync.dma_start(out=s2t[:], in_=vch[c][2:nr + 2, :])
                        sh1.append(s1t)
                        sh2.append(s2t)
                    for hc in range(2):
                        cs, ce = hc * (W // 2), (hc + 1) * (W // 2)
                        lo = max(cs - 1, 0)
                        hi = min(ce + 1, W)
                        Wl = hi - lo
                        co = cs - lo
                        dxs, dys = [], []
                        for c in range(3):
                            dx, dy = _sobel_raw(nc, pr, vch[c][0:nr, lo:hi], sh1[c][:, lo:hi],
                                                sh2[c][:, lo:hi], nr, "wsb", f"wsb_{g0}_{hc}_{c}", ctag=str(c))
                            dxs.append(dx)
                            dys.append(dy)
                        ncr = []
                        for (i1, i2) in [(1, 2), (2, 0), (0, 1)]:
                            t = pr.tile([nr, Wl], f32, tag=f"ncr{i1}", name=f"ncr_{g0}_{hc}_{i1}")
                            nc.vector.tensor_tensor(t[:], dxs[i1][:], dys[i2][:], OP.mult)
                            t2 = pr.tile([nr, Wl], f32, tag="ncrt", name=f"ncrt_{g0}_{hc}_{i1}")
                            nc.vector.tensor_tensor(t2[:], dxs[i2][:], dys[i1][:], OP.mult)
                            nc.vector.tensor_tensor(t[:], t[:], t2[:], OP.subtract)
                            ncr.append(t)
                        s1 = pr.tile([nr, Wl], f32, tag="wns", name=f"wns_{g0}_{hc}")
                        nc.vector.tensor_tensor(s1[:], ncr[0][:], ncr[0][:], OP.mult)
                        s2 = pr.tile([nr, Wl], f32, tag="wns2", name=f"wns2_{g0}_{hc}")
                        nc.vector.tensor_tensor(s2[:], ncr[1][:], ncr[1][:], OP.mult)
                        nc.vector.tensor_tensor(s1[:], s1[:], s2[:], OP.add)
                        nc.vector.tensor_tensor(s2[:], ncr[2][:], ncr[2][:], OP.mult)
                        nc.vector.tensor_tensor(s1[:], s1[:], s2[:], OP.add)
                        nc.scalar.activation(s1[:], s1[:], ACT.Sqrt)
                        nc.vector.tensor_scalar(s1[:], s1[:], 1e-8, None, OP.add)
                        nc.vector.reciprocal(s1[:], s1[:])
                        dI = sh1[2][:, lo:hi]
                        c1 = pr.tile([nr, Wl], f32, tag="wc1", name=f"wc1_{g0}_{hc}")
                        nc.vector.tensor_scalar(c1[:], dI, cst[0:nr, CD1MIN:CD1MIN + 1], None, OP.is_le)
                        c2 = pr.tile([nr, Wl], f32, tag="wc2", name=f"wc2_{g0}_{hc}")
                        nc.vector.tensor_scalar(c2[:], dI, cst[0:nr, CD1MAX:CD1MAX + 1], None, OP.is_ge)
                        nc.vector.tensor_tensor(c1[:], c1[:], c2[:], OP.max)
                        nc.vector.tensor_scalar(c1[:], c1[:], -1.0, 1.0, OP.mult, OP.add)
                        nc.vector.tensor_tensor(s1[:], s1[:], c1[:], OP.mult)
                        pixt = pr.tile([nr, (W // 2) * 9], f32, tag="pixt", name=f"pixt_{g0}_{hc}")
                        p3 = pixt[:].rearrange("p (c s) -> p c s", s=9)
                        nc.vector.tensor_copy(p3[:, :, 0], sh1[0][:, cs:ce])
                        nc.vector.tensor_copy(p3[:, :, 1], sh1[1][:, cs:ce])
                        nc.vector.tensor_copy(p3[:, :, 2], sh1[2][:, cs:ce])
                        for c in range(3):
                            nm2 = pr.tile([nr, W // 2], f32, tag="wnm", name=f"wnm_{g0}_{hc}_{c}")
                            nc.vector.tensor_tensor(nm2[:], ncr[c][:, co:co + W // 2], s1[:, co:co + W // 2], OP.mult)
                            nc.vector.tensor_copy(p3[:, :, 3 + c], nm2[:])
                        for c in range(3):
                            x1t16 = pr.tile([nr, W // 2], mybir.dt.uint8, tag="x1l", bufs=2, name=f"x1l_{g0}_{hc}_{c}")
                            nc.sync.dma_start(out=x1t16[:], in_=d_x1c.ap()[c * VW + woff:c * VW + woff + nr, cs:ce])
                            nc.vector.tensor_scalar(p3[:, :, 6 + c], x1t16[:], 1.0 / 255.0, None, OP.mult)
                        pix3 = d_pix.ap().rearrange("(r w) c -> r w c", w=W)
                        nc.sync.dma_start(out=pix3[woff:woff + nr, cs:ce, :], in_=p3[:])

            # ------------- rec4 packing (4 taps per record) --------------
            # rec4[(r,u)] = [pix(r,u), pix(r,u+1), pix(r+1,u), pix(r+1,u+1)]
            # column +1 replicates at u=W-1; row 151's +1 blocks are never
            # gathered (lv is clamped to VW-2) so source rows clamp to 151.
            with tc.tile_pool(name="pre4", bufs=1) as pr:
                rec4v = d_rec4.ap().rearrange("(r w) c -> r w c", w=W)
                pixv = d_pix.ap().rearrange("(r w) c -> r w c", w=W)
                for (rs, g) in [(0, 128), (128, VW - 128)]:
                    pb = pr.tile([g, W * 9], f32, tag="pb", name=f"pb_{rs}")
                    nc.sync.dma_start(out=pb[:], in_=pixv[rs:rs + g, :, :])
                    ps_ = pr.tile([g, W * 9], f32, tag="ps", name=f"ps_{rs}")
                    srows = [min(r + 1, VW - 1) for r in range(rs, rs + g)]
                    # contiguous ranges: rows rs+1 .. rs+g (clamped at VW-1)
                    if srows[-1] == VW - 1 and srows[0] != VW - 1:
                        ncontig = (VW - 1) - (rs + 1) + 1
                        nc.sync.dma_start(out=ps_[0:ncontig, :], in_=pixv[rs + 1:VW, :, :])
                        if ncontig < g:
                            nc.sync.dma_start(out=ps_[ncontig:g, :], in_=pixv[VW - 1:VW, :, :])
                    else:
                        nc.sync.dma_start(out=ps_[:], in_=pixv[rs + 1:rs + 1 + g, :, :])
                    pb3 = pb[:].rearrange("p (w c) -> p w c", c=9)
                    ps3 = ps_[:].rearrange("p (w c) -> p w c", c=9)
                    r4 = rec4v[rs:rs + g, :, :]
                    Q = W // 4
                    for q in range(4):
                        qs, qe = q * Q, (q + 1) * Q
                        eng = (nc.sync, nc.scalar)[q % 2]
                        eng.dma_start(out=r4[:, qs:qe, 0:9], in_=pb3[:, qs:qe, :])
                        ce2 = qe if q < 3 else W - 1
                        eng.dma_start(out=r4[:, qs:ce2, 9:18], in_=pb3[:, qs + 1:ce2 + 1, :])
                        eng.dma_start(out=r4[:, qs:qe, 18:27], in_=ps3[:, qs:qe, :])
                        eng.dma_start(out=r4[:, qs:ce2, 27:36], in_=ps3[:, qs + 1:ce2 + 1, :])
                    nc.sync.dma_start(out=r4[:, W - 1:W, 9:18], in_=pb3[:, W - 1:W, :])
                    nc.sync.dma_start(out=r4[:, W - 1:W, 27:36], in_=ps3[:, W - 1:W, :])

            # ================= iterations ================================
            with tc.tile_pool(name="wk", bufs=2) as wk:
                accv = acc3[:].rearrange("p (e k) -> p e k", k=NCHUNK)
                for it in range(MAX_ITER):
                    nc.vector.memset(acc3[:], 0.0)
                    cks = []
                    for k in range(3):
                        ck = wk.tile([BAND, 1], f32, tag=f"ck{k}", name=f"ck_{it}_{k}")
                        nc.vector.tensor_scalar(ck[:], bcol[:], pose_bc[:, 3 * k + 1:3 * k + 2], pose_bc[:, 3 * k + 2:3 * k + 3], OP.mult, OP.add)
                        cks.append(ck)

                    for cki in range(NCHUNK):
                        c0 = cki * CH
                        sl = slice(c0, c0 + CH)
                        nm = f"_{it}_{cki}"

                        def wt(tag, shape=(BAND, CH), dtype=f32, bufs=1):
                            return wk.tile(list(shape), dtype, tag=tag, bufs=bufs, name=tag + nm)

                        v0t = []
                        for k in range(3):
                            q = wt(f"q{k}")
                            nc.vector.tensor_scalar(q[:], a[0:BAND, sl], pose_bc[:, 3 * k:3 * k + 1], cks[k][:], OP.mult, OP.add)
                            v = wt(f"v0t{k}")
                            nc.vector.tensor_tensor(v[:], q[:], d0[:, sl], OP.mult)
                            nc.vector.tensor_scalar(v[:], v[:], pose_bc[:, 9 + k:10 + k], None, OP.add)
                            v0t.append(v)
                        zr = wt("zr")
                        nc.vector.reciprocal(zr[:], v0t[2][:])
                        u = wt("u")
                        nc.vector.tensor_tensor(u[:], v0t[0][:], zr[:], OP.mult)
                        nc.vector.tensor_scalar(u[:], u[:], CS(CFX), CS(CCX), OP.mult, OP.add)
                        v = wt("v")
                        nc.vector.tensor_tensor(v[:], v0t[1][:], zr[:], OP.mult)
                        nc.vector.tensor_scalar(v[:], v[:], CS(CFY), CS(CCY), OP.mult, OP.add)
                        ivw = wt("ivw")
                        nc.vector.tensor_scalar(ivw[:], u[:], 0.0, None, OP.is_gt)
                        cmp = wt("cmp")
                        nc.vector.tensor_scalar(cmp[:], u[:], float(W - 1), None, OP.is_lt)
                        nc.vector.tensor_tensor(ivw[:], ivw[:], cmp[:], OP.mult)
                        nc.vector.tensor_scalar(cmp[:], v[:], 0.0, None, OP.is_gt)
                        nc.vector.tensor_tensor(ivw[:], ivw[:], cmp[:], OP.mult)
                        nc.vector.tensor_scalar(cmp[:], v[:], float(H - 1), None, OP.is_lt)
                        nc.vector.tensor_tensor(ivw[:], ivw[:], cmp[:], OP.mult)
                        nc.vector.tensor_scalar(u[:], u[:], 0.0, float(W - 1), OP.max, OP.min)
                        nc.vector.tensor_scalar(v[:], v[:], 0.0, float(H - 1), OP.max, OP.min)
                        ui = wt("ui", dtype=i32)
                        nc.vector.tensor_copy(ui[:], u[:])
                        u0f = wt("u0f")
                        nc.vector.tensor_copy(u0f[:], ui[:])
                        nc.vector.tensor_tensor(cmp[:], u[:], u0f[:], OP.is_lt)
                        nc.vector.tensor_tensor(u0f[:], u0f[:], cmp[:], OP.subtract)
                        wu = wt("wu")
                        nc.vector.tensor_tensor(wu[:], u[:], u0f[:], OP.subtract)
                        vi = wt("vi", dtype=i32)
                        nc.vector.tensor_copy(vi[:], v[:])
                        v0f = wt("v0f")
                        nc.vector.tensor_copy(v0f[:], vi[:])
                        nc.vector.tensor_tensor(cmp[:], v[:], v0f[:], OP.is_lt)
                        nc.vector.tensor_tensor(v0f[:], v0f[:], cmp[:], OP.subtract)
                        wvv = wt("wvv")
                        nc.vector.tensor_tensor(wvv[:], v[:], v0f[:], OP.subtract)
                        lv = wt("lv")
                        nc.vector.tensor_scalar(lv[:], v0f[:], CS(CW0), None, OP.subtract)
                        nc.vector.tensor_scalar(lv[:], lv[:], 0.0, float(VW - 2), OP.max, OP.min)
                        idxf = wt("idxf")
                        nc.vector.scalar_tensor_tensor(idxf[:], lv[:], float(W), u0f[:], OP.mult, OP.add)
                        ii = wt("ii", dtype=i32, bufs=2)
                        nc.vector.tensor_copy(ii[:], idxf[:])
                        rec = wt("rec", shape=(BAND, CH * 36), bufs=2)
                        nc.gpsimd.indirect_dma_start(
                            out=rec[:], out_offset=None, in_=d_rec4.ap(),
                            in_offset=bass.IndirectOffsetOnAxis(ap=ii[:], axis=0))
                        rec3 = rec[:].rearrange("p (c s) -> p c s", s=36)
                        taps = [rec3[:, :, 9 * t:9 * t + 9] for t in range(4)]
                        omu = wt("omu")
                        nc.vector.tensor_scalar(omu[:], wu[:], -1.0, 1.0, OP.mult, OP.add)
                        omv = wt("omv")
                        nc.vector.tensor_scalar(omv[:], wvv[:], -1.0, 1.0, OP.mult, OP.add)
                        w00 = wt("w00")
                        nc.vector.tensor_tensor(w00[:], omu[:], omv[:], OP.mult)
                        w01 = wt("w01")
                        nc.vector.tensor_tensor(w01[:], wu[:], omv[:], OP.mult)
                        w10 = wt("w10")
                        nc.vector.tensor_tensor(w10[:], omu[:], wvv[:], OP.mult)
                        w11 = wt("w11")
                        nc.vector.tensor_tensor(w11[:], wu[:], wvv[:], OP.mult)
                        smp = wt("smp", shape=(BAND, CH * 9))
                        s3 = smp[:].rearrange("p (c s) -> p c s", s=9)
                        wb = [w00, w01, w10, w11]
                        tmp9 = wt("tmp9", shape=(BAND, CH * 9), bufs=1)
                        t9 = tmp9[:].rearrange("p (c s) -> p c s", s=9)
                        for ti in range(4):
                            t3 = taps[ti]
                            wbc = wb[ti][:].rearrange("p (c o) -> p c o", o=1).to_broadcast([BAND, CH, 9])
                            if ti == 0:
                                nc.vector.tensor_tensor(s3, t3, wbc, OP.mult)
                            else:
                                nc.vector.tensor_tensor(t9, t3, wbc, OP.mult)
                                nc.vector.tensor_tensor(smp[:], smp[:], tmp9[:], OP.add)
                        n1 = []
                        for k in range(3):
                            t = wt(f"n1{k}")
                            nc.vector.tensor_copy(t[:], s3[:, :, 3 + k])
                            n1.append(t)
                        dif = []
                        for k in range(3):
                            t = wt(f"dif{k}")
                            nc.vector.tensor_tensor(t[:], s3[:, :, k], v0t[k][:], OP.subtract)
                            dif.append(t)
                        res = wt("res")
                        nc.vector.tensor_tensor(res[:], n1[0][:], dif[0][:], OP.mult)
                        for k in (1, 2):
                            nc.vector.tensor_tensor(cmp[:], n1[k][:], dif[k][:], OP.mult)
                            nc.vector.tensor_tensor(res[:], res[:], cmp[:], OP.add)
                        dist2 = wt("dist2")
                        nc.vector.tensor_tensor(dist2[:], dif[0][:], dif[0][:], OP.mult)
                        for k in (1, 2):
                            nc.vector.tensor_tensor(cmp[:], dif[k][:], dif[k][:], OP.mult)
                            nc.vector.tensor_tensor(dist2[:], dist2[:], cmp[:], OP.add)
                        valid = wt("valid")
                        nc.vector.tensor_scalar(valid[:], dist2[:], DIST_THRESH * DIST_THRESH, None, OP.is_lt)
                        nc.vector.tensor_scalar(cmp[:], s3[:, :, 2], 0.0, None, OP.is_gt)
                        nc.vector.tensor_tensor(valid[:], valid[:], cmp[:], OP.mult)
                        nc.vector.tensor_tensor(valid[:], valid[:], ivw[:], OP.mult)
                        ar = wt("ar")
                        nc.scalar.activation(ar[:], res[:], ACT.Abs)
                        hs1 = wt("hs1")
                        nc.vector.tensor_scalar(hs1[:], ar[:], 2.0 * HUBER_B, -HUBER_B * HUBER_B, OP.mult, OP.add)
                        hs2 = wt("hs2")
                        nc.vector.tensor_tensor(hs2[:], ar[:], ar[:], OP.mult)
                        cmp8 = wt("cmp8", dtype=mybir.dt.uint8)
                        nc.vector.tensor_scalar(cmp8[:], ar[:], HUBER_B, None, OP.is_le)
                        rho = wt("rho")
                        nc.vector.tensor_copy(rho[:], hs1[:])
                        nc.vector.copy_predicated(rho[:], cmp8[:], hs2[:])
                        wgt = wt("wgt")
                        nc.scalar.activation(wgt[:], rho[:], ACT.Sqrt, bias=1e-16)
                        xs = wt("xs")
                        nc.vector.tensor_scalar(xs[:], ar[:], 1e-8, None, OP.max)
                        nc.vector.reciprocal(xs[:], xs[:])
                        nc.vector.tensor_tensor(wgt[:], wgt[:], xs[:], OP.mult)
                        nc.vector.tensor_tensor(wgt[:], wgt[:], valid[:], OP.mult)
                        wr = wt("wr")
                        nc.vector.tensor_tensor(wr[:], wgt[:], res[:], OP.mult)
                        wJ = []
                        for (i1, i2) in [(1, 2), (2, 0), (0, 1)]:
                            t = wt(f"cr{i1}{i2}")
                            nc.vector.tensor_tensor(t[:], v0t[i1][:], n1[i2][:], OP.mult)
                            nc.vector.tensor_tensor(cmp[:], v0t[i2][:], n1[i1][:], OP.mult)
                            nc.vector.tensor_tensor(t[:], t[:], cmp[:], OP.subtract)
                            nc.vector.tensor_tensor(t[:], t[:], wgt[:], OP.mult)
                            wJ.append(t)
                        for k in range(3):
                            t = wt(f"wn{k}")
                            nc.vector.tensor_tensor(t[:], n1[k][:], wgt[:], OP.mult)
                            wJ.append(t)
                        prods = wk.tile([BAND, NENT * CH], f32, tag="prods", bufs=1, name="prods" + nm)
                        pv = prods[:].rearrange("p (e c) -> p e c", c=CH)
                        e = 0
                        for (ai, bi) in PAIRS:
                            nc.vector.tensor_tensor(pv[:, e, :], wJ[ai][:], wJ[bi][:], OP.mult)
                            e += 1
                        for ai in range(6):
                            nc.vector.tensor_tensor(pv[:, e, :], wJ[ai][:], wr[:], OP.mult)
                            e += 1
                        rr = []
                        for c in range(3):
                            t = wt(f"rr{c}")
                            nc.vector.tensor_tensor(t[:], s3[:, :, 6 + c], x0p[c][:, sl], OP.subtract)
                            rr.append(t)
                        Pm = wt("Pm")
                        nc.vector.tensor_tensor(Pm[:], gx[0][:, sl], rr[0][:], OP.mult)
                        Qm = wt("Qm")
                        nc.vector.tensor_tensor(Qm[:], gy[0][:, sl], rr[0][:], OP.mult)
                        for c in (1, 2):
                            nc.vector.tensor_tensor(cmp[:], gx[c][:, sl], rr[c][:], OP.mult)
                            nc.vector.tensor_tensor(Pm[:], Pm[:], cmp[:], OP.add)
                            nc.vector.tensor_tensor(cmp[:], gy[c][:, sl], rr[c][:], OP.mult)
                            nc.vector.tensor_tensor(Qm[:], Qm[:], cmp[:], OP.add)
                        nc.vector.tensor_tensor(Pm[:], Pm[:], ivw[:], OP.mult)
                        nc.vector.tensor_tensor(Qm[:], Qm[:], ivw[:], OP.mult)
                        mch = wk.tile([BAND, 21 * CH], f32, tag="mch", bufs=1, name="mch" + nm)
                        nc.sync.dma_start(out=mch[:], in_=d_M.ap()[:, :, sl])
                        m3 = mch[:].rearrange("p (e c) -> p e c", c=CH)
                        for ti in range(21):
                            nc.vector.tensor_tensor(pv[:, e, :], m3[:, ti, :], ivw[:], OP.mult)
                            e += 1
                        # per-chunk jacobian helper planes
                        a2p1c = wt("a2p1c")
                        nc.vector.tensor_tensor(a2p1c[:], a[0:BAND, sl], a[0:BAND, sl], OP.mult)
                        nc.vector.tensor_scalar(a2p1c[:], a2p1c[:], 1.0, None, OP.add)
                        b2p1c = wt("b2p1c")
                        nc.vector.tensor_scalar(b2p1c[:], bpl[:, sl], bcol[:], 1.0, OP.mult, OP.add)
                        aIDc = wt("aIDc")
                        nc.vector.tensor_tensor(aIDc[:], a[0:BAND, sl], invD[:, sl], OP.mult)
                        bIDc = wt("bIDc")
                        nc.vector.tensor_scalar(bIDc[:], invD[:, sl], bcol[:], None, OP.mult)
                        planes = {"ab": ab[:, sl], "a2p1": a2p1c[:], "bpl": bpl[:, sl],
                                  "invD": invD[:, sl], "aID": aIDc[:], "b2p1": b2p1c[:],
                                  "a": a[0:BAND, sl], "bID": bIDc[:]}
                        for (pname, _, _) in P_LIST:
                            nc.vector.tensor_tensor(pv[:, e, :], planes[pname], Pm[:], OP.mult)
                            e += 1
                        for (pname, _, _) in Q_LIST:
                            nc.vector.tensor_tensor(pv[:, e, :], planes[pname], Qm[:], OP.mult)
                            e += 1
                        assert e == NENT
                        nc.vector.tensor_reduce(accv[:, :, cki], pv, mybir.AxisListType.X, OP.add)

                    # ---- reduce + AllReduce ----
                    accE = wk.tile([BAND, NENT], f32, tag="accE", name=f"accE_{it}")
                    av2 = accE[:].rearrange("p (e o) -> p e o", o=1)
                    nc.vector.tensor_reduce(av2, accv, mybir.AxisListType.X, OP.add)
                    tps = pps.tile([NENT, BAND], f32, tag="tps")
                    nc.tensor.transpose(out=tps[:], in_=accE[:], identity=ident[:])
                    gcol = wk.tile([NENT, 1], f32, tag="gcol", name=f"gcol_{it}")
                    nc.vector.tensor_reduce(gcol[:], tps[:], mybir.AxisListType.X, OP.add)
                    gpad = wk.tile([1, 64], f32, tag="gpad", name=f"gpad_{it}")
                    nc.vector.memset(gpad[:], 0.0)
                    nc.sync.dma_start(out=gpad[:, 0:NENT], in_=gcol[:])
                    nc.sync.dma_start(out=d_gram.ap(), in_=gpad[:])
                    nc.gpsimd.collective_compute(
                        "AllReduce", OP.add, replica_groups=groups,
                        ins=[d_gram.ap()], outs=[d_gramS.ap()])
                    nc.sync.dma_start(out=gram_sb[:], in_=d_gramS.ap())

                    # ---- assemble augmented H [6,7] ----
                    ctri = wk.tile([1, 21], f32, tag="ctri", name=f"ctri_{it}")
                    nc.vector.tensor_scalar(ctri[:], gram_sb[:, 27:48], LAMBDA_RGB, None, OP.mult)
                    nc.vector.tensor_tensor(ctri[:], ctri[:], gram_sb[:, 0:21], OP.add)
                    stage = wk.tile([1, 42], f32, tag="stage", name=f"stage_{it}")
                    nc.vector.memset(stage[:], 0.0)
                    for aa in range(6):
                        off = 6 * aa - aa * (aa - 1) // 2
                        nc.vector.tensor_copy(stage[:, 7 * aa + aa:7 * aa + 6], ctri[:, off:off + 6 - aa])
                    for aa in range(6):
                        for bb in range(aa):
                            off = 6 * bb - bb * (bb - 1) // 2 + (aa - bb)
                            nc.vector.tensor_copy(stage[:, 7 * aa + bb:7 * aa + bb + 1], ctri[:, off:off + 1])
                    for k, (pname, slot, sg) in enumerate(P_LIST):
                        nc.vector.scalar_tensor_tensor(
                            stage[:, 7 * slot + 6:7 * slot + 7], gram_sb[:, 48 + k:49 + k],
                            sg * LAMBDA_RGB, stage[:, 7 * slot + 6:7 * slot + 7], OP.mult, OP.add)
                    for k, (pname, slot, sg) in enumerate(Q_LIST):
                        nc.vector.scalar_tensor_tensor(
                            stage[:, 7 * slot + 6:7 * slot + 7], gram_sb[:, 53 + k:54 + k],
                            sg * LAMBDA_RGB, stage[:, 7 * slot + 6:7 * slot + 7], OP.mult, OP.add)
                    rhsv = stage[:].rearrange("p (r c) -> p r c", c=7)[:, :, 6]
                    nc.vector.tensor_tensor(rhsv, rhsv, gram_sb[:, 21:27], OP.add)
                    dg = wk.tile([1, 6], f32, tag="dg", name=f"dg_{it}")
                    for aa in range(6):
                        nc.vector.tensor_copy(dg[:, aa:aa + 1], stage[:, 7 * aa + aa:7 * aa + aa + 1])
                    tr = wk.tile([1, 1], f32, tag="tr", name=f"tr_{it}")
                    nc.vector.tensor_reduce(tr[:], dg[:], mybir.AxisListType.X, OP.add)
                    for aa in range(6):
                        nc.vector.scalar_tensor_tensor(
                            stage[:, 7 * aa + aa:7 * aa + aa + 1], tr[:], DAMPING,
                            stage[:, 7 * aa + aa:7 * aa + aa + 1], OP.mult, OP.add)
                    Haug = wk.tile([6, 7], f32, tag="Haug", name=f"Haug_{it}")
                    nc.sync.dma_start(out=Haug[:], in_=stage[:])
                    # ---- Gauss-Jordan ----
                    dval = wk.tile([6, 1], f32, tag="dval", name=f"dval_{it}")
                    for k in range(6):
                        prow0 = wk.tile([1, 7], f32, tag="prow0", name=f"prow0_{it}_{k}")
                        nc.sync.dma_start(out=prow0[:], in_=Haug[k:k + 1, :])
                        prow_ps = pps.tile([6, 7], f32, tag="prowps")
                        nc.tensor.matmul(out=prow_ps[:], lhsT=ones6[:], rhs=prow0[:])
                        prow = wk.tile([6, 7], f32, tag="prow", name=f"prow_{it}_{k}")
                        nc.vector.tensor_copy(prow[:], prow_ps[:])
                        nc.sync.dma_start(out=dval[k:k + 1, :], in_=prow0[:, k:k + 1])
                        rp = wk.tile([6, 1], f32, tag="rp", name=f"rp_{it}_{k}")
                        nc.vector.reciprocal(rp[:], prow[:, k:k + 1])
                        fac = wk.tile([6, 1], f32, tag="fac", name=f"fac_{it}_{k}")
                        nc.vector.tensor_tensor(fac[:], Haug[:, k:k + 1], rp[:], OP.mult)
                        nc.vector.tensor_tensor(fac[:], fac[:], cst[0:6, CNEG + k:CNEG + k + 1], OP.mult)
                        nc.vector.scalar_tensor_tensor(Haug[:], prow[:], fac[:], Haug[:], OP.mult, OP.add)
                    xi6 = wk.tile([6, 1], f32, tag="xi6", name=f"xi6_{it}")
                    nc.vector.reciprocal(xi6[:], dval[:])
                    nc.vector.tensor_tensor(xi6[:], xi6[:], Haug[:, 6:7], OP.mult)
                    xif = wk.tile([1, 6], f32, tag="xif", name=f"xif_{it}")
                    nc.sync.dma_start(out=xif[:], in_=xi6[:])
                    # ---- exp_so3 (series) + pose update ----
                    wn = wk.tile([1, 3], f32, tag="wn", name=f"wn_{it}")
                    nc.vector.tensor_scalar(wn[:], xif[:, 0:3], -1.0, None, OP.mult)
                    sq = wk.tile([1, 3], f32, tag="sq3", name=f"sq3_{it}")
                    nc.vector.tensor_tensor(sq[:], wn[:], wn[:], OP.mult)
                    th2 = wk.tile([1, 1], f32, tag="th2", name=f"th2_{it}")
                    nc.vector.tensor_reduce(th2[:], sq[:], mybir.AxisListType.X, OP.add)
                    th4 = wk.tile([1, 1], f32, tag="th4", name=f"th4_{it}")
                    nc.vector.tensor_tensor(th4[:], th2[:], th2[:], OP.mult)
                    Acf = wk.tile([1, 1], f32, tag="Acf", name=f"Acf_{it}")
                    nc.vector.tensor_scalar(Acf[:], th2[:], -1.0 / 6.0, 1.0, OP.mult, OP.add)
                    nc.vector.scalar_tensor_tensor(Acf[:], th4[:], 1.0 / 120.0, Acf[:], OP.mult, OP.add)
                    Bcf = wk.tile([1, 1], f32, tag="Bcf", name=f"Bcf_{it}")
                    nc.vector.tensor_scalar(Bcf[:], th2[:], -1.0 / 24.0, 0.5, OP.mult, OP.add)
                    nc.vector.scalar_tensor_tensor(Bcf[:], th4[:], 1.0 / 720.0, Bcf[:], OP.mult, OP.add)
                    nAcf = wk.tile([1, 1], f32, tag="nAcf", name=f"nAcf_{it}")
                    nc.vector.tensor_scalar(nAcf[:], Acf[:], -1.0, None, OP.mult)
                    Wf = wk.tile([1, 9], f32, tag="Wf", name=f"Wf_{it}")
                    nc.vector.memset(Wf[:], 0.0)
                    nc.vector.tensor_scalar(Wf[:, 1:2], wn[:, 2:3], -1.0, None, OP.mult)
                    nc.vector.tensor_copy(Wf[:, 2:3], wn[:, 1:2])
                    nc.vector.tensor_copy(Wf[:, 3:4], wn[:, 2:3])
                    nc.vector.tensor_scalar(Wf[:, 5:6], wn[:, 0:1], -1.0, None, OP.mult)
                    nc.vector.tensor_scalar(Wf[:, 6:7], wn[:, 1:2], -1.0, None, OP.mult)
                    nc.vector.tensor_copy(Wf[:, 7:8], wn[:, 0:1])
                    W33 = wk.tile([3, 3], f32, tag="W33", name=f"W33_{it}")
                    nc.sync.dma_start(out=W33[:], in_=Wf[:])
                    W33n = wk.tile([3, 3], f32, tag="W33n", name=f"W33n_{it}")
                    nc.vector.tensor_scalar(W33n[:], W33[:], -1.0, None, OP.mult)
                    wsq_ps = pps.tile([3, 3], f32, tag="wsqps")
                    nc.tensor.matmul(out=wsq_ps[:], lhsT=W33n[:], rhs=W33[:])
                    Wsq = wk.tile([3, 3], f32, tag="Wsq", name=f"Wsq_{it}")
                    nc.vector.tensor_copy(Wsq[:], wsq_ps[:])
                    Wsqf = wk.tile([1, 9], f32, tag="Wsqf", name=f"Wsqf_{it}")
                    nc.sync.dma_start(out=Wsqf[:], in_=Wsq[:])
                    dRTf = wk.tile([1, 9], f32, tag="dRTf", name=f"dRTf_{it}")
                    nc.vector.tensor_scalar(dRTf[:], Wf[:], nAcf[:, 0:1], None, OP.mult)
                    nc.vector.scalar_tensor_tensor(dRTf[:], Wsqf[:], Bcf[:, 0:1], dRTf[:], OP.mult, OP.add)
                    nc.vector.tensor_tensor(dRTf[:], dRTf[:], cst[0:1, CEYE9:CEYE9 + 9], OP.add)
                    dRT = wk.tile([3, 3], f32, tag="dRT", name=f"dRT_{it}")
                    nc.sync.dma_start(out=dRT[:], in_=dRTf[:])
                    tv = wk.tile([1, 3], f32, tag="tv", name=f"tv_{it}")
                    nc.vector.tensor_tensor(tv[:], poseflat[:, 9:12], xif[:, 3:6], OP.subtract)
                    tv31 = wk.tile([3, 1], f32, tag="tv31", name=f"tv31_{it}")
                    nc.sync.dma_start(out=tv31[:], in_=tv[:])
                    r1_ps = pps.tile([3, 3], f32, tag="r1ps")
                    nc.tensor.matmul(out=r1_ps[:], lhsT=dRT[:], rhs=Rp33[:])
                    R1 = wk.tile([3, 3], f32, tag="R1", name=f"R1_{it}")
                    nc.vector.tensor_copy(R1[:], r1_ps[:])
                    t1_ps = pps.tile([3, 1], f32, tag="t1ps")
                    nc.tensor.matmul(out=t1_ps[:], lhsT=dRT[:], rhs=tv31[:])
                    t1 = wk.tile([3, 1], f32, tag="t1", name=f"t1_{it}")
                    nc.vector.tensor_copy(t1[:], t1_ps[:])
                    nc.vector.tensor_copy(Rp33[:], R1[:])
                    nc.sync.dma_start(out=poseflat[:, 0:9], in_=R1[:])
                    nc.sync.dma_start(out=poseflat[:, 9:12], in_=t1[:])
                    pose_ps2 = pps.tile([BAND, 12], f32, tag="poseps")
                    nc.tensor.matmul(out=pose_ps2[:], lhsT=ones120[:], rhs=poseflat[:])
                    nc.vector.tensor_copy(pose_bc[:], pose_ps2[:])

                # ------------- output ------------------------------------
                outt = wk.tile([1, 64], f32, tag="outt")
                nc.vector.memset(outt[:], 0.0)
                nc.vector.tensor_copy(outt[:, 0:12], poseflat[:])
                nc.vector.tensor_copy(outt[:, 16:48], gram_sb[:, 0:32])
                nc.sync.dma_start(out=d_out.ap(), in_=outt[:])

    import bass_rust as _bass_rust
    _bass_rust.generate_event_semaphores(nc)
    return nc


# ===================== host-side preparation =========================
def prepare_inputs(pose10, depth0, depth1, x0, x1, K):
